# revision 1
# baseline (speedup 1.0000x reference)
"""Trainium2 Bass kernel for CHAI sparse attention (nn_Attention_28235115004180).

8-core SPMD, cluster-parallel sharding:
  - Core c owns score cluster c: only the 8 rep heads' q/k are ever used
    by the reference, so each core computes just its rep head's q/k
    (RoPE applied via a host-baked pair-swap matmul), transposed scores
    exp(k_dT.T @ q_dT + mask) with 1/sqrt(HD) folded into wq, the member
    heads' V projection, and AV with an appended ones-column that yields
    the softmax denominator (no max-subtraction: scores are bounded).
  - Attention outputs are transposed on-chip and AllGathered (two
    sequence-half collectives, pipelined against compute), then each
    core computes 512 output columns with a host-pretransposed,
    head-permuted wo shard. The host only slices/transposes/concats;
    all FLOPs run on device.

kernel(**inputs) takes the FULL unsharded inputs and returns the FULL
[1, 1024, 4096] float32 output.
"""

import math
import numpy as np

import concourse.bass as bass
import concourse.bacc as bacc
import concourse.tile as tile
import concourse.mybir as mybir
from concourse.masks import make_identity

P = 128
S = 1024
D = 4096
HD = 128
H = 32
C = 8
NB = D // P        # 32 blocks over the D contraction
SB = S // P        # 8 blocks over sequence
OC = 512           # output columns per core
NCORES = 8
NEG = -1e9
SH = 512           # sequence slice width (2 pipelined AGs)

F32 = mybir.dt.float32
BF16 = mybir.dt.bfloat16


# ----------------------------------------------------------------- host prep

def prep(inputs, compute_in_bf16_io=True):
    x = np.asarray(inputs["x"], np.float32)
    wq = np.asarray(inputs["wq"], np.float32)
    wk = np.asarray(inputs["wk"], np.float32)
    wv = np.asarray(inputs["wv"], np.float32)
    wo = np.asarray(inputs["wo"], np.float32)
    cos = np.asarray(inputs["freqs_cos"], np.float32)   # [S, HD/2]
    sin = np.asarray(inputs["freqs_sin"], np.float32)
    mask = np.asarray(inputs["mask"], np.float32)       # [S, S]
    lab = np.asarray(inputs["cluster_assignment"]).astype(np.int64)
    rep = np.asarray(inputs["rep_heads"]).astype(np.int64)

    members = [[h for h in range(H) if lab[h] == c] for c in range(C)]
    maxn = max(len(m) for m in members)
    W = maxn * P

    causal_mask = np.where(np.triu(np.ones((S, S), bool), k=1),
                           np.float32(NEG), np.float32(0.0))
    causal = np.array_equal(mask, causal_mask)

    import ml_dtypes
    bf = ml_dtypes.bfloat16

    def cvt(a):
        if compute_in_bf16_io:
            return np.ascontiguousarray(a).astype(bf)
        return np.ascontiguousarray(a, dtype=np.float32)

    xT = cvt(x[0].T)                                    # [D, S]

    cs = np.empty((HD, S), np.float32)
    sn = np.empty((HD, S), np.float32)
    cs[0::2] = cos.T
    cs[1::2] = cos.T
    sn[0::2] = -sin.T
    sn[1::2] = sin.T

    psw = np.zeros((HD, HD), np.float32)
    for i in range(HD):
        psw[i, i ^ 1] = 1.0

    kk = np.arange(P)[:, None]
    qq = np.arange(P)[None, :]
    tri = np.where(kk > qq, np.float32(NEG), np.float32(0.0))

    # block order for the gathered attention / wo permutation
    blocks = [(c, j) for c in range(C) for j in range(len(members[c]))]
    assert len(blocks) == H
    ag_rows = [c * maxn + j for (c, j) in blocks]

    scale = 1.0 / math.sqrt(HD)
    in_maps = []
    for c in range(NCORES):
        wqT = cvt(wq[rep[c] * HD:(rep[c] + 1) * HD, :].T * scale)
        wkT = cvt(wk[rep[c] * HD:(rep[c] + 1) * HD, :].T)
        wvT = np.zeros((D, W), np.float32)
        for j, h in enumerate(members[c]):
            wvT[:, j * P:(j + 1) * P] = wv[h * HD:(h + 1) * HD, :].T
        wvT = cvt(wvT)
        woT = np.empty((H * P, OC), np.float32)
        for r, (cc, j) in enumerate(blocks):
            h = members[cc][j]
            woT[r * P:(r + 1) * P, :] = wo[c * OC:(c + 1) * OC,
                                           h * HD:(h + 1) * HD].T
        woT = cvt(woT)
        m = {
            "xT": xT, "wqT": wqT, "wkT": wkT, "wvT": wvT, "woT": woT,
            "cs": cs, "sn": sn, "psw": psw, "tri": tri,
        }
        if not causal:
            m["maskT"] = np.ascontiguousarray(mask.T)
        in_maps.append(m)

    meta = {
        "maxn": maxn, "W": W, "causal": causal, "ag_rows": ag_rows,
        "nper": [len(m) for m in members],
        "bf16_io": compute_in_bf16_io,
    }
    return in_maps, meta


def assemble(results, meta):
    out = np.empty((1, S, D), np.float32)
    for c in range(NCORES):
        out[0, :, c * OC:(c + 1) * OC] = results[c]["out"]
    return out


# ------------------------------------------------------------- kernel build

def emit_body(nc, tc, tensors, meta, body_idx=0, no_collective=False,
              chain=False):
    maxn, W, causal = meta["maxn"], meta["W"], meta["causal"]
    nper = meta["nper"]
    xT, wqT, wkT, wvT, woT = (tensors[k] for k in
                              ("xT", "wqT", "wkT", "wvT", "woT"))
    cs, sn, psw, tri = (tensors[k] for k in ("cs", "sn", "psw", "tri"))
    out = tensors["out"]
    maskT = tensors.get("maskT")

    WP1 = W + 1
    av_chunks = [(0, min(512, WP1))]
    if WP1 > 512:
        av_chunks.append((512, WP1))
    v_chunks = [(0, min(512, W))]
    if W > 512:
        v_chunks.append((512, W))

    with (
        tc.tile_pool(name="const", bufs=1) as constp,
        tc.tile_pool(name="persist", bufs=1) as persist,
    ):
        ident = constp.tile([P, P], BF16)
        make_identity(nc, ident[:])
        pswf = constp.tile([P, P], F32)
        nc.sync.dma_start(pswf[:], psw[:])
        pswb = constp.tile([P, P], BF16)
        nc.any.tensor_copy(pswb[:], pswf[:])
        csb = constp.tile([P, S], F32)
        nc.sync.dma_start(csb[:], cs[:])
        snb = constp.tile([P, S], F32)
        nc.sync.dma_start(snb[:], sn[:])
        trib = constp.tile([P, P], F32)
        nc.sync.dma_start(trib[:], tri[:])

        q_dT = persist.tile([P, S], BF16)
        k_dT = persist.tile([P, S], BF16)
        Vs = [persist.tile([P, WP1], BF16, name=f"V{kb}_{body_idx}")
              for kb in range(SB)]
        aT = [persist.tile([P, maxn, SH], BF16, name=f"aT{h}_{body_idx}")
              for h in range(S // SH)]
        pTs = [persist.tile([P, S], BF16, name=f"pT{kb}_{body_idx}")
               for kb in range(SB)]

        junk_b = None
        if chain and body_idx > 0:
            # timing-only: serialize bodies on the previous body's output
            junk = constp.tile([P, 16], F32, name="junk")
            nc.sync.dma_start(junk[:], out[0:P, 0:16])
            junk_b = constp.tile([P, 16], BF16, name="junkb")
            nc.vector.tensor_scalar_mul(junk_b[:], junk[:], 0.0)

        def chain_dep(ap2d):
            if junk_b is not None:
                nc.vector.tensor_copy(ap2d, junk_b[:])

        # ---------------- phase 1: projections -------------------------
        XG = 8
        GB = NB // XG
        with (
            tc.tile_pool(name="ph1", bufs=2) as ph1,
            tc.tile_pool(name="ph1w", bufs=1) as ph1w,
            tc.tile_pool(name="psum1", bufs=2, space="PSUM") as psum1,
        ):
            xbs = []
            for g in range(XG):
                xg = ph1w.tile([P, GB, S], BF16, name=f"xb{g}")
                chain_dep(xg[:, 0, :16])
                nc.sync.dma_start(
                    xg[:],
                    xT[g * GB * P:(g + 1) * GB * P, :]
                    .rearrange("(b p) f -> p b f", p=P))
                xbs.append(xg)

            def xb(db):
                return xbs[db // GB][:, db % GB, :]

            wqb = ph1w.tile([P, NB, HD], BF16)
            wkb = ph1w.tile([P, NB, HD], BF16)
            for (wt, dst) in ((wqT, wqb), (wkT, wkb)):
                chain_dep(dst[:, 0, :16])
                nc.sync.dma_start(
                    dst[:], wt[:].rearrange("(b p) h -> p b h", p=P))
            wvbs = []
            for g in range(XG):
                wvg = ph1w.tile([P, GB, W], BF16, name=f"wvb{g}")
                chain_dep(wvg[:, 0, :16])
                nc.sync.dma_start(
                    wvg[:],
                    wvT[g * GB * P:(g + 1) * GB * P, :]
                    .rearrange("(b p) w -> p b w", p=P))
                wvbs.append(wvg)

            def wvb(db):
                return wvbs[db // GB][:, db % GB, :]

            # q/k projections [d, s] + RoPE via swap-matmul
            for wt, dst in ((wqb, q_dT), (wkb, k_dT)):
                raw = ph1.tile([P, S], BF16, tag="qraw")
                for ch in range(2):
                    ps = psum1.tile([P, 512], F32, tag="pqk")
                    for db in range(NB):
                        nc.tensor.matmul(
                            ps[:], wt[:, db, :],
                            xb(db)[:, ch * 512:(ch + 1) * 512],
                            start=(db == 0), stop=(db == NB - 1))
                    nc.any.tensor_copy(raw[:, ch * 512:(ch + 1) * 512], ps[:])
                for ch in range(2):
                    csl = slice(ch * 512, (ch + 1) * 512)
                    ps2 = psum1.tile([P, 512], F32, tag="pswp")
                    nc.tensor.matmul(ps2[:], pswb[:], raw[:, csl],
                                     start=True, stop=True)
                    t1 = ph1.tile([P, 512], F32, tag="rope1")
                    nc.vector.tensor_tensor(t1[:], ps2[:], snb[:, csl],
                                            op=mybir.AluOpType.mult)
                    t2 = ph1.tile([P, 512], F32, tag="rope2")
                    nc.vector.tensor_tensor(t2[:], raw[:, csl], csb[:, csl],
                                            op=mybir.AluOpType.mult)
                    nc.vector.tensor_tensor(dst[:, csl], t1[:], t2[:],
                                            op=mybir.AluOpType.add)

            # V projection [s, vd] (+ ones column at W)
            for sb in range(SB):
                for (c0, c1) in v_chunks:
                    ps = psum1.tile([P, 512], F32, tag="pv")
                    for db in range(NB):
                        nc.tensor.matmul(
                            ps[:, :c1 - c0],
                            xb(db)[:, sb * P:(sb + 1) * P],
                            wvb(db)[:, c0:c1],
                            start=(db == 0), stop=(db == NB - 1))
                    nc.any.tensor_copy(Vs[sb][:, c0:c1], ps[:, :c1 - c0])
                nc.vector.memset(Vs[sb][:, W:W + 1], 1.0)

            # scores + exp (overlaps the V projection; q/k ready early)
            for kb in range(SB):
                q0 = kb * P if causal else 0
                if maskT is not None:
                    mrow = ph1.tile([P, S], F32, tag="mrow")
                    nc.sync.dma_start(mrow[:], maskT[kb * P:(kb + 1) * P, :])
                c0 = q0
                while c0 < S:
                    c1 = min(c0 + 512, S)
                    ps = psum1.tile([P, 512], F32, tag="psc")
                    nc.tensor.matmul(ps[:, :c1 - c0],
                                     k_dT[:, kb * P:(kb + 1) * P],
                                     q_dT[:, c0:c1], start=True, stop=True)
                    if maskT is not None:
                        nc.vector.tensor_tensor(
                            ps[:, :c1 - c0], ps[:, :c1 - c0], mrow[:, c0:c1],
                            op=mybir.AluOpType.add)
                    elif causal and c0 == q0:
                        nc.vector.tensor_tensor(
                            ps[:, :P], ps[:, :P], trib[:],
                            op=mybir.AluOpType.add)
                    nc.scalar.activation(pTs[kb][:, c0:c1], ps[:, :c1 - c0],
                                         mybir.ActivationFunctionType.Exp)
                    c0 = c1

        # ---- phases 2..5 -----------------------------------------------
        with (
            tc.tile_pool(name="ph5w", bufs=1) as ph5w,
            tc.tile_pool(name="ph2", bufs=2) as ph2,
            tc.tile_pool(name="ph5", bufs=4) as ph5,
            tc.tile_pool(name="pav", bufs=1, space="PSUM") as pavp,
            tc.tile_pool(name="ptr", bufs=2, space="PSUM") as ptrp,
            tc.tile_pool(name="po", bufs=1, space="PSUM") as pop,
            tc.tile_pool(name="dram", bufs=1, space="DRAM") as dram,
        ):
            # prefetch the (permuted) output weights while attention runs
            wob = ph5w.tile([P, H, OC], BF16)
            nc.sync.dma_start(
                wob[:], woT[:].rearrange("(b p) o -> p b o", p=P))

            # -------- phase 3: AV + normalize + transpose ---------------
            for qb in range(SB):
                kmax = qb + 1 if causal else SB
                pss = []
                for (c0, c1) in av_chunks:
                    i = len(pss)
                    pss.append(pavp.tile([P, c1 - c0], F32, tag=f"pav{i}",
                                         name=f"pav{i}_{qb}_{body_idx}"))
                for ki in range(kmax):
                    lt = pTs[ki][:, qb * P:(qb + 1) * P]
                    for (cc, (c0, c1)) in zip(pss, av_chunks):
                        nc.tensor.matmul(cc[:], lt, Vs[ki][:, c0:c1],
                                         start=(ki == 0), stop=(ki == kmax - 1))
                rL = ph2.tile([P, 1], F32, tag="rL")
                nc.vector.reciprocal(rL[:], pss[-1][:, -1:])
                attn = ph2.tile([P, W], BF16, tag="attn")
                for (cc, (c0, c1)) in zip(pss, av_chunks):
                    w1 = min(c1, W)
                    if w1 > c0:
                        # normalize on ScalarE (Copy with per-partition
                        # scale) so DVE isn't on the transpose path
                        nc.scalar.activation(
                            attn[:, c0:w1], cc[:, :w1 - c0],
                            mybir.ActivationFunctionType.Copy,
                            scale=rL[:])
                qb_per = SH // P
                h, sl = qb // qb_per, (qb % qb_per) * P
                for blk in range(maxn):
                    pst = ptrp.tile([P, P], BF16, tag="ptr")
                    nc.tensor.transpose(pst[:], attn[:, blk * P:(blk + 1) * P],
                                        ident[:])
                    nc.any.tensor_copy(aT[h][:, blk, sl:sl + P], pst[:])

            # -------- phases 4+5: per-half AllGather + wo matmul --------
            bounces, agouts = [], []
            for h in range(S // SH):
                b = dram.tile([maxn * P, SH], BF16,
                              name=f"bounce{body_idx}_{h}")
                g = dram.tile([NCORES * maxn * P, SH], BF16,
                              addr_space="Local" if no_collective else "Shared",
                              name=f"agout{body_idx}_{h}")
                bounces.append(b)
                agouts.append(g)
                nc.sync.dma_start(
                    b[:].rearrange("(b p) f -> p b f", p=P), aT[h][:])
                if no_collective:
                    for cc in range(NCORES):
                        nc.sync.dma_start(
                            g[cc * maxn * P:(cc + 1) * maxn * P, :], b[:])
                else:
                    nc.gpsimd.collective_compute(
                        "AllGather", mybir.AluOpType.bypass,
                        replica_groups=[list(range(NCORES))],
                        ins=[b[:].opt()], outs=[g[:].opt()])

            NR = len(meta["ag_rows"])
            nsb = SH // P
            for h in range(S // SH):
                psums = [pop.tile([P, OC], F32, tag=f"po{i}",
                                  name=f"po{i}_{h}_{body_idx}")
                         for i in range(nsb)]
                r = 0
                for cc in range(NCORES):
                    ncc = nper[cc]
                    agt = ph5.tile([P, maxn, SH], BF16, tag="agt",
                                   name=f"agt{h}_{cc}_{body_idx}")
                    nc.sync.dma_start(
                        agt[:, :ncc, :],
                        agouts[h][cc * maxn * P:(cc * maxn + ncc) * P, :]
                        .rearrange("(b p) f -> p b f", p=P))
                    for j in range(ncc):
                        for i in range(nsb):
                            nc.tensor.matmul(
                                psums[i][:],
                                agt[:, j, i * P:(i + 1) * P],
                                wob[:, r, :],
                                start=(r == 0), stop=(r == NR - 1))
                        r += 1
                otile = ph5.tile([P, nsb, OC], F32, tag="ot",
                                 name=f"ot{h}_{body_idx}")
                for i in range(nsb):
                    nc.any.tensor_copy(otile[:, i, :], psums[i][:])
                nc.sync.dma_start(
                    out[h * SH:(h + 1) * SH, :]
                    .rearrange("(b p) o -> p b o", p=P), otile[:])


def build_kernel(meta, repeat=1, chain=True):
    nc = bacc.Bacc("TRN2", target_bir_lowering=False, debug=False,
                   num_devices=NCORES)
    in_dt = BF16 if meta["bf16_io"] else F32
    W = meta["W"]
    tensors = {
        "xT": nc.dram_tensor("xT", [D, S], in_dt, kind="ExternalInput"),
        "wqT": nc.dram_tensor("wqT", [D, HD], in_dt, kind="ExternalInput"),
        "wkT": nc.dram_tensor("wkT", [D, HD], in_dt, kind="ExternalInput"),
        "wvT": nc.dram_tensor("wvT", [D, W], in_dt, kind="ExternalInput"),
        "woT": nc.dram_tensor("woT", [H * P, OC], in_dt, kind="ExternalInput"),
        "cs": nc.dram_tensor("cs", [HD, S], F32, kind="ExternalInput"),
        "sn": nc.dram_tensor("sn", [HD, S], F32, kind="ExternalInput"),
        "psw": nc.dram_tensor("psw", [HD, HD], F32, kind="ExternalInput"),
        "tri": nc.dram_tensor("tri", [P, P], F32, kind="ExternalInput"),
        "out": nc.dram_tensor("out", [S, OC], F32, kind="ExternalOutput"),
    }
    if not meta["causal"]:
        tensors["maskT"] = nc.dram_tensor("maskT", [S, S], F32,
                                          kind="ExternalInput")
    with tile.TileContext(nc) as tc:
        if repeat == 0:
            with tc.tile_pool(name="z", bufs=1) as zp:
                zt = zp.tile([P, SB, OC], F32)
                nc.vector.memset(zt[:], 0.0)
                nc.sync.dma_start(
                    tensors["out"][:].rearrange("(b p) o -> p b o", p=P),
                    zt[:])
        else:
            for r in range(repeat):
                emit_body(nc, tc, tensors, meta, body_idx=r, chain=chain)
    nc.compile()
    return nc


# --------------------------------------------------- dual-slot (v4) variant

def _decompose_31(nper):
    """Split clusters into 8 three-head chunks + 8 one-head chunks
    (each core = one 3-chunk + one 1-chunk). Returns (threes, ones)
    as cluster-id lists of length 8, or None if impossible."""
    a = [n // 3 for n in nper]
    total = sum(a)
    if total < 8:
        return None
    # reduce surplus threes (each -1 three frees 3 ones)
    cs_ = list(range(len(nper)))
    i = 0
    while total > 8:
        c = cs_[i % len(cs_)]
        if a[c] > 0:
            a[c] -= 1
            total -= 1
        i += 1
    b = [nper[c] - 3 * a[c] for c in range(len(nper))]
    if sum(b) != 8 or any(x < 0 for x in b):
        return None
    threes, ones = [], []
    for c in range(len(nper)):
        threes += [c] * a[c]
        ones += [c] * b[c]
    if len(threes) != 8 or len(ones) != 8:
        return None
    return threes, ones


def prep_dual(inputs, np_mod=np):
    """Host prep for the dual-slot layout. Returns (in_maps, meta) or None."""
    x = np.asarray(inputs["x"], np.float32)
    wq = np.asarray(inputs["wq"], np.float32)
    wk = np.asarray(inputs["wk"], np.float32)
    wv = np.asarray(inputs["wv"], np.float32)
    wo = np.asarray(inputs["wo"], np.float32)
    cos = np.asarray(inputs["freqs_cos"], np.float32)
    sin = np.asarray(inputs["freqs_sin"], np.float32)
    mask = np.asarray(inputs["mask"], np.float32)
    lab = np.asarray(inputs["cluster_assignment"]).astype(np.int64)
    rep = np.asarray(inputs["rep_heads"]).astype(np.int64)

    members = [[h for h in range(H) if lab[h] == c] for c in range(C)]
    nper = [len(m) for m in members]
    dec = _decompose_31(nper)
    if dec is None:
        return None
    threes, ones = dec

    # assign heads to chunks, consuming each cluster's member list in order
    cursor = {c: 0 for c in range(C)}

    def take(c, k):
        i0 = cursor[c]
        cursor[c] += k
        return members[c][i0:i0 + k]

    slotsA = [(c, take(c, 3)) for c in threes]
    slotsB = [(c, take(c, 1)) for c in ones]
    assert all(cursor[c] == nper[c] for c in range(C))

    causal_mask = np.where(np.triu(np.ones((S, S), bool), k=1),
                           np.float32(NEG), np.float32(0.0))
    causal = np.array_equal(mask, causal_mask)

    import ml_dtypes
    bf = ml_dtypes.bfloat16

    def cvt(a):
        return np.ascontiguousarray(a).astype(bf)

    xT = cvt(x[0].T)
    cs = np.empty((HD, S), np.float32)
    sn = np.empty((HD, S), np.float32)
    cs[0::2] = cos.T
    cs[1::2] = cos.T
    sn[0::2] = -sin.T
    sn[1::2] = sin.T
    psw = np.zeros((HD, HD), np.float32)
    for i in range(HD):
        psw[i, i ^ 1] = 1.0
    kk = np.arange(P)[:, None]
    qq = np.arange(P)[None, :]
    tri = np.where(kk > qq, np.float32(NEG), np.float32(0.0))

    # global gathered block order: rank i contributes [A0,A1,A2,B0]
    scale = 1.0 / math.sqrt(HD)
    all_heads = []
    for i in range(NCORES):
        all_heads += slotsA[i][1] + slotsB[i][1]
    assert sorted(all_heads) == list(range(H))

    in_maps = []
    for i in range(NCORES):
        cA, headsA = slotsA[i]
        cB, headsB = slotsB[i]
        wqk = np.empty((D, 4 * HD), np.float32)
        wqk[:, 0 * HD:1 * HD] = wq[rep[cA] * HD:(rep[cA] + 1) * HD, :].T * scale
        wqk[:, 1 * HD:2 * HD] = wk[rep[cA] * HD:(rep[cA] + 1) * HD, :].T
        wqk[:, 2 * HD:3 * HD] = wq[rep[cB] * HD:(rep[cB] + 1) * HD, :].T * scale
        wqk[:, 3 * HD:4 * HD] = wk[rep[cB] * HD:(rep[cB] + 1) * HD, :].T
        wvT = np.empty((D, 4 * HD), np.float32)
        for j, h in enumerate(headsA + headsB):
            wvT[:, j * P:(j + 1) * P] = wv[h * HD:(h + 1) * HD, :].T
        woT = np.empty((H * P, OC), np.float32)
        for r, h in enumerate(all_heads):
            woT[r * P:(r + 1) * P, :] = wo[i * OC:(i + 1) * OC,
                                           h * HD:(h + 1) * HD].T
        m = {
            "xT": xT, "wqkT": cvt(wqk), "wvT": cvt(wvT), "woT": cvt(woT),
            "cs": cs, "sn": sn, "psw": psw, "tri": tri,
        }
        if not causal:
            m["maskT"] = np.ascontiguousarray(mask.T)
        in_maps.append(m)

    meta = {"causal": causal, "dual": True, "bf16_io": True}
    return in_maps, meta


def emit_body_dual(nc, tc, tensors, meta, body_idx=0, chain=False):
    causal = meta["causal"]
    xT, wqkT, wvT, woT = (tensors[k] for k in ("xT", "wqkT", "wvT", "woT"))
    cs, sn, psw, tri = (tensors[k] for k in ("cs", "sn", "psw", "tri"))
    out = tensors["out"]
    maskT = tensors.get("maskT")

    # V_sb columns: [VA 0:384 | onesA 384 | VB 385:513 | onesB 513]
    WA, WB = 384, 128
    OA, OB = WA, WA + 1 + WB          # ones col offsets: 384, 513
    VW = WA + WB + 2                   # 514

    with (
        tc.tile_pool(name="const", bufs=1) as constp,
        tc.tile_pool(name="persist", bufs=1) as persist,
    ):
        ident = constp.tile([P, P], BF16)
        make_identity(nc, ident[:])
        pswf = constp.tile([P, P], F32)
        nc.sync.dma_start(pswf[:], psw[:])
        pswb = constp.tile([P, P], BF16)
        nc.any.tensor_copy(pswb[:], pswf[:])
        csb = constp.tile([P, S], F32)
        nc.sync.dma_start(csb[:], cs[:])
        snb = constp.tile([P, S], F32)
        nc.sync.dma_start(snb[:], sn[:])
        trib = constp.tile([P, P], F32)
        nc.sync.dma_start(trib[:], tri[:])

        qk_dT = [persist.tile([P, S], BF16, name=f"qk{i}_{body_idx}")
                 for i in range(4)]    # qA, kA, qB, kB
        Vs = [persist.tile([P, VW], BF16, name=f"V{kb}_{body_idx}")
              for kb in range(SB)]
        aT = [persist.tile([P, 4, SH], BF16, name=f"aT{h}_{body_idx}")
              for h in range(S // SH)]
        pTa = [persist.tile([P, S], BF16, name=f"pTa{kb}_{body_idx}")
               for kb in range(SB)]
        pTb = [persist.tile([P, S], BF16, name=f"pTb{kb}_{body_idx}")
               for kb in range(SB)]

        junk_b = None
        if chain and body_idx > 0:
            junk = constp.tile([P, 16], F32, name="junk")
            nc.sync.dma_start(junk[:], out[0:P, 0:16])
            junk_b = constp.tile([P, 16], BF16, name="junkb")
            nc.vector.tensor_scalar_mul(junk_b[:], junk[:], 0.0)

        def chain_dep(ap2d):
            if junk_b is not None:
                nc.vector.tensor_copy(ap2d, junk_b[:])

        # ---------------- phase 1: projections + scores ----------------
        XG = 4
        GB = NB // XG
        with (
            tc.tile_pool(name="ph1", bufs=2) as ph1,
            tc.tile_pool(name="ph1w", bufs=1) as ph1w,
            tc.tile_pool(name="psum1", bufs=2, space="PSUM") as psum1,
        ):
            xbs = []
            for g in range(XG):
                xg = ph1w.tile([P, GB, S], BF16, name=f"xb{g}")
                chain_dep(xg[:, 0, :16])
                nc.sync.dma_start(
                    xg[:],
                    xT[g * GB * P:(g + 1) * GB * P, :]
                    .rearrange("(b p) f -> p b f", p=P))
                xbs.append(xg)

            def xb(db):
                return xbs[db // GB][:, db % GB, :]

            wqkb = ph1w.tile([P, NB, 4 * HD], BF16)
            chain_dep(wqkb[:, 0, :16])
            nc.sync.dma_start(
                wqkb[:], wqkT[:].rearrange("(b p) h -> p b h", p=P))
            wvb = ph1w.tile([P, NB, 4 * HD], BF16)
            chain_dep(wvb[:, 0, :16])
            nc.sync.dma_start(
                wvb[:], wvT[:].rearrange("(b p) w -> p b w", p=P))

            # 4 projections [d, s] + RoPE via swap-matmul
            for idx in range(4):
                dst = qk_dT[idx]
                raw = ph1.tile([P, S], BF16, tag="qraw")
                for ch in range(2):
                    ps = psum1.tile([P, 512], F32, tag="pqk")
                    for db in range(NB):
                        nc.tensor.matmul(
                            ps[:], wqkb[:, db, idx * HD:(idx + 1) * HD],
                            xb(db)[:, ch * 512:(ch + 1) * 512],
                            start=(db == 0), stop=(db == NB - 1))
                    nc.any.tensor_copy(raw[:, ch * 512:(ch + 1) * 512], ps[:])
                for ch in range(2):
                    csl = slice(ch * 512, (ch + 1) * 512)
                    ps2 = psum1.tile([P, 512], F32, tag="pswp")
                    nc.tensor.matmul(ps2[:], pswb[:], raw[:, csl],
                                     start=True, stop=True)
                    t1 = ph1.tile([P, 512], F32, tag="rope1")
                    nc.vector.tensor_tensor(t1[:], ps2[:], snb[:, csl],
                                            op=mybir.AluOpType.mult)
                    t2 = ph1.tile([P, 512], F32, tag="rope2")
                    nc.vector.tensor_tensor(t2[:], raw[:, csl], csb[:, csl],
                                            op=mybir.AluOpType.mult)
                    nc.vector.tensor_tensor(dst[:, csl], t1[:], t2[:],
                                            op=mybir.AluOpType.add)

            # V projection (one 512-wide matmul per (sb, db))
            for sb in range(SB):
                ps = psum1.tile([P, 512], F32, tag="pv")
                for db in range(NB):
                    nc.tensor.matmul(
                        ps[:], xb(db)[:, sb * P:(sb + 1) * P],
                        wvb[:, db, :],
                        start=(db == 0), stop=(db == NB - 1))
                nc.any.tensor_copy(Vs[sb][:, 0:WA], ps[:, 0:WA])
                nc.any.tensor_copy(Vs[sb][:, OA + 1:OA + 1 + WB],
                                   ps[:, WA:WA + WB])
                nc.vector.memset(Vs[sb][:, OA:OA + 1], 1.0)
                nc.vector.memset(Vs[sb][:, OB:OB + 1], 1.0)

            # scores + exp for both groups (overlap the V projection)
            for grp, (qi, ki_, pTs) in enumerate(((0, 1, pTa), (2, 3, pTb))):
                for kb in range(SB):
                    q0 = kb * P if causal else 0
                    if maskT is not None:
                        mrow = ph1.tile([P, S], F32, tag="mrow")
                        nc.sync.dma_start(mrow[:],
                                          maskT[kb * P:(kb + 1) * P, :])
                    c0 = q0
                    while c0 < S:
                        c1 = min(c0 + 512, S)
                        ps = psum1.tile([P, 512], F32, tag="psc")
                        nc.tensor.matmul(ps[:, :c1 - c0],
                                         qk_dT[ki_][:, kb * P:(kb + 1) * P],
                                         qk_dT[qi][:, c0:c1],
                                         start=True, stop=True)
                        if maskT is not None:
                            nc.vector.tensor_tensor(
                                ps[:, :c1 - c0], ps[:, :c1 - c0],
                                mrow[:, c0:c1], op=mybir.AluOpType.add)
                        elif causal and c0 == q0:
                            nc.vector.tensor_tensor(
                                ps[:, :P], ps[:, :P], trib[:],
                                op=mybir.AluOpType.add)
                        nc.scalar.activation(
                            pTs[kb][:, c0:c1], ps[:, :c1 - c0],
                            mybir.ActivationFunctionType.Exp)
                        c0 = c1

        # ---- phases 3..5 ----------------------------------------------
        with (
            tc.tile_pool(name="ph5w", bufs=1) as ph5w,
            tc.tile_pool(name="ph2", bufs=2) as ph2,
            tc.tile_pool(name="ph5", bufs=2) as ph5,
            tc.tile_pool(name="pava", bufs=1, space="PSUM") as pavap,
            tc.tile_pool(name="pavb", bufs=1, space="PSUM") as pavbp,
            tc.tile_pool(name="ptr", bufs=1, space="PSUM") as ptrp,
            tc.tile_pool(name="po", bufs=1, space="PSUM") as pop,
            tc.tile_pool(name="dram", bufs=1, space="DRAM") as dram,
        ):
            wob = ph5w.tile([P, H, OC], BF16)
            nc.sync.dma_start(
                wob[:], woT[:].rearrange("(b p) o -> p b o", p=P))

            # -------- phase 3: AV + normalize + transpose ---------------
            for qb in range(SB):
                kmax = qb + 1 if causal else SB
                psA = pavap.tile([P, WA + 1], F32, tag="pava",
                                 name=f"pava{qb}_{body_idx}")
                psB = pavbp.tile([P, WB + 1], F32, tag="pavb",
                                 name=f"pavb{qb}_{body_idx}")
                for ki in range(kmax):
                    ltA = pTa[ki][:, qb * P:(qb + 1) * P]
                    ltB = pTb[ki][:, qb * P:(qb + 1) * P]
                    nc.tensor.matmul(psA[:], ltA, Vs[ki][:, 0:WA + 1],
                                     start=(ki == 0), stop=(ki == kmax - 1))
                    nc.tensor.matmul(psB[:], ltB,
                                     Vs[ki][:, OA + 1:OB + 1],
                                     start=(ki == 0), stop=(ki == kmax - 1))
                rLA = ph2.tile([P, 1], F32, tag="rLA")
                nc.vector.reciprocal(rLA[:], psA[:, WA:WA + 1])
                rLB = ph2.tile([P, 1], F32, tag="rLB")
                nc.vector.reciprocal(rLB[:], psB[:, WB:WB + 1])
                attn = ph2.tile([P, 512], BF16, tag="attn")
                nc.vector.tensor_scalar_mul(attn[:, 0:WA], psA[:, :WA],
                                            rLA[:])
                nc.vector.tensor_scalar_mul(attn[:, WA:512], psB[:, :WB],
                                            rLB[:])
                h, sl = qb // (SH // P), (qb % (SH // P)) * P
                for blk in range(4):
                    pst = ptrp.tile([P, P], BF16, tag="ptr")
                    nc.tensor.transpose(pst[:], attn[:, blk * P:(blk + 1) * P],
                                        ident[:])
                    nc.any.tensor_copy(aT[h][:, blk, sl:sl + P], pst[:])

            # -------- phases 4+5: per-half AllGather + wo matmul --------
            agouts = []
            for h in range(S // SH):
                b = dram.tile([4 * P, SH], BF16, name=f"bounce{body_idx}_{h}")
                g = dram.tile([NCORES * 4 * P, SH], BF16, addr_space="Shared",
                              name=f"agout{body_idx}_{h}")
                agouts.append(g)
                nc.sync.dma_start(
                    b[:].rearrange("(b p) f -> p b f", p=P), aT[h][:])
                nc.gpsimd.collective_compute(
                    "AllGather", mybir.AluOpType.bypass,
                    replica_groups=[list(range(NCORES))],
                    ins=[b[:].opt()], outs=[g[:].opt()])

            nsb = SH // P
            for h in range(S // SH):
                psums = [pop.tile([P, OC], F32, tag=f"po{i}",
                                  name=f"po{i}_{h}_{body_idx}")
                         for i in range(nsb)]
                agt = ph5.tile([P, H, SH], BF16, tag="agt",
                               name=f"agt{h}_{body_idx}")
                nc.sync.dma_start(
                    agt[:], agouts[h][:].rearrange("(b p) f -> p b f", p=P))
                for r in range(H):
                    for i in range(nsb):
                        nc.tensor.matmul(
                            psums[i][:], agt[:, r, i * P:(i + 1) * P],
                            wob[:, r, :], start=(r == 0), stop=(r == H - 1))
                otile = ph5.tile([P, nsb, OC], F32, tag="ot",
                                 name=f"ot{h}_{body_idx}")
                for i in range(nsb):
                    nc.any.tensor_copy(otile[:, i, :], psums[i][:])
                nc.sync.dma_start(
                    out[h * SH:(h + 1) * SH, :]
                    .rearrange("(b p) o -> p b o", p=P), otile[:])


def build_kernel_dual(meta, repeat=1, chain=True):
    nc = bacc.Bacc("TRN2", target_bir_lowering=False, debug=False,
                   num_devices=NCORES)
    tensors = {
        "xT": nc.dram_tensor("xT", [D, S], BF16, kind="ExternalInput"),
        "wqkT": nc.dram_tensor("wqkT", [D, 4 * HD], BF16,
                               kind="ExternalInput"),
        "wvT": nc.dram_tensor("wvT", [D, 4 * HD], BF16, kind="ExternalInput"),
        "woT": nc.dram_tensor("woT", [H * P, OC], BF16, kind="ExternalInput"),
        "cs": nc.dram_tensor("cs", [HD, S], F32, kind="ExternalInput"),
        "sn": nc.dram_tensor("sn", [HD, S], F32, kind="ExternalInput"),
        "psw": nc.dram_tensor("psw", [HD, HD], F32, kind="ExternalInput"),
        "tri": nc.dram_tensor("tri", [P, P], F32, kind="ExternalInput"),
        "out": nc.dram_tensor("out", [S, OC], F32, kind="ExternalOutput"),
    }
    if not meta["causal"]:
        tensors["maskT"] = nc.dram_tensor("maskT", [S, S], F32,
                                          kind="ExternalInput")
    with tile.TileContext(nc) as tc:
        if repeat == 0:
            with tc.tile_pool(name="z", bufs=1) as zp:
                zt = zp.tile([P, SB, OC], F32)
                nc.vector.memset(zt[:], 0.0)
                nc.sync.dma_start(
                    tensors["out"][:].rearrange("(b p) o -> p b o", p=P),
                    zt[:])
        else:
            for r in range(repeat):
                emit_body_dual(nc, tc, tensors, meta, body_idx=r, chain=chain)
    nc.compile()
    return nc


# ----------------------------------------------------------- SPMD entry point

def kernel(**inputs):
    import numpy as _np
    np_inputs = {k: (_np.asarray(v) if not _np.isscalar(v) else v)
                 for k, v in inputs.items()}
    in_maps, meta = prep(np_inputs, compute_in_bf16_io=True)
    nc = build_kernel(meta, repeat=1, chain=False)
    from concourse import bass_utils
    res = bass_utils.run_bass_kernel_spmd(
        nc, in_maps, core_ids=list(range(NCORES)))
    return assemble(res.results, meta)



# revision 2
# speedup vs baseline: 1.1703x; 1.1703x over previous
"""Trainium2 Bass kernel v3 for CHAI sparse attention.

Per-core layout (8-core SPMD, one NEFF):
  - dual-slot balanced heads: each core owns a 3-head chunk of cluster
    cA plus a 1-head chunk of cluster cB (every core exactly 4 heads,
    V-projection width 512). This keeps the attention AllGather at
    0.5 MB/rank per half, below the slow-algorithm cliff measured at
    0.75 MB/rank (80 us vs 20 us per AllGather).
  - all projections local (qA,kA,qB,kB); the A-pair rides the x DMA
    wave (weights DMA'd first), exp(scores_A) on ACT overlaps the
    B-pair projections on PE.
  - per-sb interleave: V(sb) + scores_B(kb=sb) + exp + AV(sb) +
    transpose; attention output AllGathered per sequence half as soon
    as its rows are done; wo matmul per half behind each AG.
"""

import math
import numpy as np

import concourse.bass as bass
import concourse.bacc as bacc
import concourse.tile as tile
import concourse.mybir as mybir
from concourse.masks import make_identity

P = 128
S = 1024
D = 4096
HD = 128
H = 32
C = 8
NB = D // P        # 32 blocks over the D contraction
SB = S // P        # 8 blocks over sequence
OC = 512           # output columns per core
NCORES = 8
NEG = -1e9
QW = 512           # AG chunk width over sequence (2 halves)
NQ = S // QW
SBQ = QW // P      # sb blocks per AG chunk

WA = 384           # A-slot attn width (3 heads)
WB = 128           # B-slot attn width
OA = WA            # ones column for A at 384
OB = WA + 1 + WB   # ones column for B at 513
VW = WA + WB + 2   # 514

F32 = mybir.dt.float32
BF16 = mybir.dt.bfloat16


def _decompose_31(nper):
    """Split clusters into 8 three-head chunks + 8 one-head chunks."""
    a = [n // 3 for n in nper]
    total = sum(a)
    if total < 8:
        return None
    cs_ = list(range(len(nper)))
    i = 0
    while total > 8:
        c = cs_[i % len(cs_)]
        if a[c] > 0:
            a[c] -= 1
            total -= 1
        i += 1
    b = [nper[c] - 3 * a[c] for c in range(len(nper))]
    if sum(b) != 8 or any(x < 0 for x in b):
        return None
    threes, ones = [], []
    for c in range(len(nper)):
        threes += [c] * a[c]
        ones += [c] * b[c]
    if len(threes) != 8 or len(ones) != 8:
        return None
    return threes, ones


def prep(inputs, compute_in_bf16_io=True):
    x = np.asarray(inputs["x"], np.float32)
    wq = np.asarray(inputs["wq"], np.float32)
    wk = np.asarray(inputs["wk"], np.float32)
    wv = np.asarray(inputs["wv"], np.float32)
    wo = np.asarray(inputs["wo"], np.float32)
    cos = np.asarray(inputs["freqs_cos"], np.float32)
    sin = np.asarray(inputs["freqs_sin"], np.float32)
    mask = np.asarray(inputs["mask"], np.float32)
    lab = np.asarray(inputs["cluster_assignment"]).astype(np.int64)
    rep = np.asarray(inputs["rep_heads"]).astype(np.int64)

    members = [[h for h in range(H) if lab[h] == c] for c in range(C)]
    nper = [len(m) for m in members]
    dec = _decompose_31(nper)
    assert dec is not None, "3+1 decomposition failed for this input"
    threes, ones = dec

    cursor = {c: 0 for c in range(C)}

    def take(c, k):
        i0 = cursor[c]
        cursor[c] += k
        return members[c][i0:i0 + k]

    slotsA = [(c, take(c, 3)) for c in threes]
    slotsB = [(c, take(c, 1)) for c in ones]
    assert all(cursor[c] == nper[c] for c in range(C))

    causal_mask = np.where(np.triu(np.ones((S, S), bool), k=1),
                           np.float32(NEG), np.float32(0.0))
    causal = np.array_equal(mask, causal_mask)

    import ml_dtypes
    bf = ml_dtypes.bfloat16

    def cvt(a):
        return np.ascontiguousarray(a).astype(bf)

    xT = cvt(x[0].T)
    cs = np.empty((HD, S), np.float32)
    sn = np.empty((HD, S), np.float32)
    cs[0::2] = cos.T
    cs[1::2] = cos.T
    sn[0::2] = -sin.T
    sn[1::2] = sin.T
    psw = np.zeros((HD, HD), np.float32)
    for i in range(HD):
        psw[i, i ^ 1] = 1.0
    kk = np.arange(P)[:, None]
    qq = np.arange(P)[None, :]
    tri = np.where(kk > qq, np.float32(NEG), np.float32(0.0))

    # global gathered block order: rank i contributes [A0,A1,A2,B0]
    scale = 1.0 / math.sqrt(HD)
    all_heads = []
    for i in range(NCORES):
        all_heads += slotsA[i][1] + slotsB[i][1]
    assert sorted(all_heads) == list(range(H))

    in_maps = []
    for i in range(NCORES):
        cA, headsA = slotsA[i]
        cB, headsB = slotsB[i]
        wqk = np.empty((D, 4 * HD), np.float32)
        wqk[:, 0 * HD:1 * HD] = wq[rep[cA] * HD:(rep[cA] + 1) * HD, :].T * scale
        wqk[:, 1 * HD:2 * HD] = wk[rep[cA] * HD:(rep[cA] + 1) * HD, :].T
        wqk[:, 2 * HD:3 * HD] = wq[rep[cB] * HD:(rep[cB] + 1) * HD, :].T * scale
        wqk[:, 3 * HD:4 * HD] = wk[rep[cB] * HD:(rep[cB] + 1) * HD, :].T
        wvT = np.empty((D, 4 * HD), np.float32)
        for j, h in enumerate(headsA + headsB):
            wvT[:, j * P:(j + 1) * P] = wv[h * HD:(h + 1) * HD, :].T
        woT = np.empty((H * P, OC), np.float32)
        for r, h in enumerate(all_heads):
            woT[r * P:(r + 1) * P, :] = wo[i * OC:(i + 1) * OC,
                                           h * HD:(h + 1) * HD].T
        m = {
            "xT": xT, "wqkT": cvt(wqk), "wvT": cvt(wvT), "woT": cvt(woT),
            "cs": cs, "sn": sn, "psw": psw, "tri": tri,
        }
        if not causal:
            m["maskT"] = np.ascontiguousarray(mask.T)
        in_maps.append(m)

    meta = {"causal": causal, "bf16_io": True}
    return in_maps, meta


def assemble(results, meta):
    out = np.empty((1, S, D), np.float32)
    for c in range(NCORES):
        out[0, :, c * OC:(c + 1) * OC] = results[c]["out"]
    return out


def emit_body(nc, tc, tensors, meta, body_idx=0, chain=False):
    causal = meta["causal"]
    xT, wqkT, wvT, woT = (tensors[k] for k in ("xT", "wqkT", "wvT", "woT"))
    cs, sn, psw, tri = (tensors[k] for k in ("cs", "sn", "psw", "tri"))
    out = tensors["out"]
    maskT = tensors.get("maskT")

    XG = 8
    GB = NB // XG

    with (
        tc.tile_pool(name="const", bufs=1) as constp,
        tc.tile_pool(name="persist", bufs=1) as persist,
        tc.tile_pool(name="ph1w", bufs=1) as ph1w,
        tc.tile_pool(name="dram", bufs=1, space="DRAM") as dram,
    ):
        ident = constp.tile([P, P], BF16)
        make_identity(nc, ident[:])
        trib = constp.tile([P, P], F32)
        nc.sync.dma_start(trib[:], tri[:])

        qk_dT = [persist.tile([P, S], BF16, name=f"qk{i}_{body_idx}")
                 for i in range(4)]    # qA, kA, qB, kB
        Vs = [persist.tile([P, VW], BF16, name=f"V{kb}_{body_idx}")
              for kb in range(SB)]
        # pT tiles shrunk to the causal width (col c maps to q = kb*P + c)
        pTw = [(S - kb * P if causal else S) for kb in range(SB)]
        pTa = [persist.tile([P, pTw[kb]], BF16, name=f"pTa{kb}_{body_idx}")
               for kb in range(SB)]
        pTb = [persist.tile([P, pTw[kb]], BF16, name=f"pTb{kb}_{body_idx}")
               for kb in range(SB)]

        junk_b = None
        if chain and body_idx > 0:
            junk = constp.tile([P, 16], F32, name="junk")
            nc.sync.dma_start(junk[:], out[0:P, 0:16])
            junk_b = constp.tile([P, 16], BF16, name="junkb")
            nc.vector.tensor_scalar_mul(junk_b[:], junk[:], 0.0)

        def chain_dep(ap2d):
            if junk_b is not None:
                nc.vector.tensor_copy(ap2d, junk_b[:])

        # ================= projections (scoped pools) ===================
        with (
            tc.tile_pool(name="wqkp", bufs=1) as wqkp,
            tc.tile_pool(name="ropep", bufs=2) as ropep,
            tc.tile_pool(name="psproj", bufs=1, space="PSUM") as psprojp,
            tc.tile_pool(name="psrope", bufs=2, space="PSUM") as psropep,
        ):
            wqkb = wqkp.tile([P, NB, 4 * HD], BF16)
            chain_dep(wqkb[:, 0, :16])
            nc.sync.dma_start(
                wqkb[:, :, 0:2 * HD],
                wqkT[:, 0:2 * HD].rearrange("(b p) h -> p b h", p=P))
            xbs = []
            for g in range(XG):
                xg = ph1w.tile([P, GB, S], BF16, name=f"xb{g}")
                chain_dep(xg[:, 0, :16])
                nc.sync.dma_start(
                    xg[:],
                    xT[g * GB * P:(g + 1) * GB * P, :]
                    .rearrange("(b p) f -> p b f", p=P))
                xbs.append(xg)
            nc.sync.dma_start(
                wqkb[:, :, 2 * HD:4 * HD],
                wqkT[:, 2 * HD:4 * HD].rearrange("(b p) h -> p b h", p=P))
            wvb = ph1w.tile([P, NB, 4 * HD], BF16)
            chain_dep(wvb[:, 0, :16])
            nc.sync.dma_start(
                wvb[:], wvT[:].rearrange("(b p) w -> p b w", p=P))
            csb = ropep.tile([P, S], F32, tag="csb")
            nc.sync.dma_start(csb[:], cs[:])
            snb = ropep.tile([P, S], F32, tag="snb")
            nc.sync.dma_start(snb[:], sn[:])
            pswf = ropep.tile([P, P], F32, tag="pswf")
            nc.sync.dma_start(pswf[:], psw[:])
            pswb = ropep.tile([P, P], BF16, tag="pswb")
            nc.vector.tensor_copy(pswb[:], pswf[:])

            def xb(db):
                return xbs[db // GB][:, db % GB, :]

            def proj_pair(base):
                # base=0 -> qA,kA ; base=2 -> qB,kB. db-outer so the
                # matmuls start as soon as the first x group lands.
                pss = [psprojp.tile([P, 512], F32, tag=f"pp{j}",
                                    name=f"pp{base}_{j}_{body_idx}")
                       for j in range(4)]  # q ch0, q ch1, k ch0, k ch1
                for db in range(NB):
                    for pj in range(2):       # q, k
                        for ch in range(2):
                            nc.tensor.matmul(
                                pss[pj * 2 + ch][:],
                                wqkb[:, db,
                                     (base + pj) * HD:(base + pj + 1) * HD],
                                xb(db)[:, ch * 512:(ch + 1) * 512],
                                start=(db == 0), stop=(db == NB - 1))
                return pss

            def rope(pss, base):
                for pj in range(2):
                    dst = qk_dT[base + pj]
                    raw = ropep.tile([P, S], BF16, tag="qraw")
                    for ch in range(2):
                        nc.vector.tensor_copy(raw[:, ch * 512:(ch + 1) * 512],
                                              pss[pj * 2 + ch][:])
                    for ch in range(2):
                        csl = slice(ch * 512, (ch + 1) * 512)
                        ps2 = psropep.tile([P, 512], F32, tag="pswp")
                        nc.tensor.matmul(ps2[:], pswb[:], raw[:, csl],
                                         start=True, stop=True)
                        t1 = ropep.tile([P, 512], F32, tag="rope1")
                        nc.vector.tensor_tensor(t1[:], ps2[:], snb[:, csl],
                                                op=mybir.AluOpType.mult)
                        t2 = ropep.tile([P, 512], F32, tag="rope2")
                        nc.vector.tensor_tensor(t2[:], raw[:, csl],
                                                csb[:, csl],
                                                op=mybir.AluOpType.mult)
                        nc.vector.tensor_tensor(dst[:, csl], t1[:], t2[:],
                                                op=mybir.AluOpType.add)

            pssA = proj_pair(0)
            rope(pssA, 0)
            pssB = proj_pair(2)
            rope(pssB, 2)

        # ============== scores_A + interleaved V/scores_B/AV ============
        agouts = [None] * NQ
        with (
            tc.tile_pool(name="ph3", bufs=2) as ph3,
            tc.tile_pool(name="aTp", bufs=2) as aTp,
            tc.tile_pool(name="psv", bufs=2, space="PSUM") as psvp,
            tc.tile_pool(name="pssc", bufs=2, space="PSUM") as psscp,
            tc.tile_pool(name="pav", bufs=1, space="PSUM") as pavp,
            tc.tile_pool(name="ptr", bufs=2, space="PSUM") as ptrp,
        ):
            def scores(qi, ki_, pTs, kb):
                q0 = kb * P if causal else 0
                if maskT is not None:
                    mrow = ph3.tile([P, S], F32, tag="mrow")
                    nc.sync.dma_start(mrow[:], maskT[kb * P:(kb + 1) * P, :])
                c0 = q0
                while c0 < S:
                    c1 = min(c0 + 512, S)
                    ps = psscp.tile([P, 512], F32, tag="psc")
                    nc.tensor.matmul(ps[:, :c1 - c0],
                                     qk_dT[ki_][:, kb * P:(kb + 1) * P],
                                     qk_dT[qi][:, c0:c1],
                                     start=True, stop=True)
                    if maskT is not None:
                        nc.vector.tensor_tensor(
                            ps[:, :c1 - c0], ps[:, :c1 - c0],
                            mrow[:, c0:c1], op=mybir.AluOpType.add)
                    elif causal and c0 == q0:
                        nc.vector.tensor_tensor(
                            ps[:, :P], ps[:, :P], trib[:],
                            op=mybir.AluOpType.add)
                    nc.scalar.activation(
                        pTs[kb][:, c0 - (kb * P if causal else 0):
                                c1 - (kb * P if causal else 0)],
                        ps[:, :c1 - c0],
                        mybir.ActivationFunctionType.Exp)
                    c0 = c1

            for kb in range(SB):
                scores(0, 1, pTa, kb)

            aTt = None
            for sb in range(SB):
                # V projection for this sequence block
                psv = psvp.tile([P, 512], F32, tag="pv")
                for db in range(NB):
                    nc.tensor.matmul(
                        psv[:], xb(db)[:, sb * P:(sb + 1) * P],
                        wvb[:, db, :],
                        start=(db == 0), stop=(db == NB - 1))
                nc.vector.tensor_copy(Vs[sb][:, 0:WA], psv[:, 0:WA])
                nc.vector.tensor_copy(Vs[sb][:, OA + 1:OA + 1 + WB],
                                      psv[:, WA:WA + WB])
                nc.vector.memset(Vs[sb][:, OA:OA + 1], 1.0)
                nc.vector.memset(Vs[sb][:, OB:OB + 1], 1.0)

                # B scores for this k block
                scores(2, 3, pTb, sb)

                # AV for q block sb (causal: ki <= sb)
                if sb % SBQ == 0:
                    aTt = aTp.tile([P, 4, QW], BF16, tag="aT",
                                   name=f"aT{sb // SBQ}_{body_idx}")
                kmax = sb + 1 if causal else SB
                psA = pavp.tile([P, WA + 1], F32, tag="pava",
                                name=f"pava{sb}_{body_idx}")
                psB = pavp.tile([P, WB + 1], F32, tag="pavb",
                                name=f"pavb{sb}_{body_idx}")
                for ki in range(kmax):
                    o = ki * P if causal else 0
                    ltA = pTa[ki][:, sb * P - o:(sb + 1) * P - o]
                    ltB = pTb[ki][:, sb * P - o:(sb + 1) * P - o]
                    nc.tensor.matmul(psA[:], ltA, Vs[ki][:, 0:WA + 1],
                                     start=(ki == 0), stop=(ki == kmax - 1))
                    nc.tensor.matmul(psB[:], ltB, Vs[ki][:, OA + 1:OB + 1],
                                     start=(ki == 0), stop=(ki == kmax - 1))
                rLA = ph3.tile([P, 1], F32, tag="rLA")
                nc.vector.reciprocal(rLA[:], psA[:, WA:WA + 1])
                rLB = ph3.tile([P, 1], F32, tag="rLB")
                nc.vector.reciprocal(rLB[:], psB[:, WB:WB + 1])
                attn = ph3.tile([P, 512], BF16, tag="attn")
                nc.vector.tensor_scalar_mul(attn[:, 0:WA], psA[:, :WA],
                                            rLA[:])
                nc.vector.tensor_scalar_mul(attn[:, WA:512], psB[:, :WB],
                                            rLB[:])
                qtr, sl = sb // SBQ, (sb % SBQ) * P
                for blk in range(4):
                    pst = ptrp.tile([P, P], BF16, tag="ptr")
                    nc.tensor.transpose(pst[:], attn[:, blk * P:(blk + 1) * P],
                                        ident[:])
                    nc.vector.tensor_copy(aTt[:, blk, sl:sl + P], pst[:])

                # kick off the AllGather as soon as a chunk is complete
                if sb % SBQ == SBQ - 1:
                    b = dram.tile([4 * P, QW], BF16,
                                  name=f"bounce{body_idx}_{qtr}")
                    g = dram.tile([NCORES * 4 * P, QW], BF16,
                                  addr_space="Shared",
                                  name=f"agout{body_idx}_{qtr}")
                    agouts[qtr] = g
                    nc.sync.dma_start(
                        b[:].rearrange("(b p) f -> p b f", p=P), aTt[:])
                    nc.gpsimd.collective_compute(
                        "AllGather", mybir.AluOpType.bypass,
                        replica_groups=[list(range(NCORES))],
                        ins=[b[:].opt()], outs=[g[:].opt()])

        # ================= wo matmul per AG chunk =======================
        with (
            tc.tile_pool(name="ph5w", bufs=1) as ph5w,
            tc.tile_pool(name="ph5", bufs=1) as ph5,
            tc.tile_pool(name="po", bufs=1, space="PSUM") as pop,
        ):
            wob = ph5w.tile([P, H, OC], BF16)
            nc.sync.dma_start(
                wob[:], woT[:].rearrange("(b p) o -> p b o", p=P))

            for qtr in range(NQ):
                g = agouts[qtr]
                agts = []
                for hf in range(2):
                    agt = ph5.tile([P, H // 2, QW], BF16, tag=f"agt{hf}",
                                   name=f"agt{qtr}_{hf}_{body_idx}")
                    nc.sync.dma_start(
                        agt[:],
                        g[hf * (H // 2) * P:(hf + 1) * (H // 2) * P, :]
                        .rearrange("(b p) f -> p b f", p=P))
                    agts.append(agt)
                nsb = QW // P
                pos = [pop.tile([P, OC], F32, tag=f"po{i}",
                                name=f"po{i}_{qtr}_{body_idx}")
                       for i in range(nsb)]
                for r in range(H):
                    agt = agts[r // (H // 2)]
                    rr = r % (H // 2)
                    for i in range(nsb):
                        nc.tensor.matmul(
                            pos[i][:], agt[:, rr, i * P:(i + 1) * P],
                            wob[:, r, :], start=(r == 0), stop=(r == H - 1))
                otile = ph5.tile([P, nsb, OC], F32, tag="ot",
                                 name=f"ot{qtr}_{body_idx}")
                for i in range(nsb):
                    nc.vector.tensor_copy(otile[:, i, :], pos[i][:])
                nc.sync.dma_start(
                    out[qtr * QW:(qtr + 1) * QW, :]
                    .rearrange("(b p) o -> p b o", p=P), otile[:])


def build_kernel(meta, repeat=1, chain=True):
    nc = bacc.Bacc("TRN2", target_bir_lowering=False, debug=False,
                   num_devices=NCORES)
    tensors = {
        "xT": nc.dram_tensor("xT", [D, S], BF16, kind="ExternalInput"),
        "wqkT": nc.dram_tensor("wqkT", [D, 4 * HD], BF16,
                               kind="ExternalInput"),
        "wvT": nc.dram_tensor("wvT", [D, 4 * HD], BF16, kind="ExternalInput"),
        "woT": nc.dram_tensor("woT", [H * P, OC], BF16, kind="ExternalInput"),
        "cs": nc.dram_tensor("cs", [HD, S], F32, kind="ExternalInput"),
        "sn": nc.dram_tensor("sn", [HD, S], F32, kind="ExternalInput"),
        "psw": nc.dram_tensor("psw", [HD, HD], F32, kind="ExternalInput"),
        "tri": nc.dram_tensor("tri", [P, P], F32, kind="ExternalInput"),
        "out": nc.dram_tensor("out", [S, OC], F32, kind="ExternalOutput"),
    }
    if not meta["causal"]:
        tensors["maskT"] = nc.dram_tensor("maskT", [S, S], F32,
                                          kind="ExternalInput")
    with tile.TileContext(nc) as tc:
        if repeat == 0:
            with tc.tile_pool(name="z", bufs=1) as zp:
                zt = zp.tile([P, SB, OC], F32)
                nc.vector.memset(zt[:], 0.0)
                nc.sync.dma_start(
                    tensors["out"][:].rearrange("(b p) o -> p b o", p=P),
                    zt[:])
        else:
            for r in range(repeat):
                emit_body(nc, tc, tensors, meta, body_idx=r, chain=chain)
    nc.compile()
    return nc


def kernel(**inputs):
    import numpy as _np
    np_inputs = {k: (_np.asarray(v) if not _np.isscalar(v) else v)
                 for k, v in inputs.items()}
    in_maps, meta = prep(np_inputs)
    nc = build_kernel(meta, repeat=1, chain=False)
    from concourse import bass_utils
    res = bass_utils.run_bass_kernel_spmd(
        nc, in_maps, core_ids=list(range(NCORES)))
    return assemble(res.results, meta)
